# revision 41
# baseline (speedup 1.0000x reference)
"""Trainium2 Bass kernel for AdaBiDiff GNN message passing.

Data parallel over batch B=8, one batch element per core.  Per core:
  xt (12,1536) -> softmax over t -> p, logp (t-major)
  kl[i,j] = rowterm[i] - sum_t p[i,t] logp[j,t];  A = (kl < 0.5)
  u_fwd = rownorm(A) @ xt.T;  u_bwd = rownorm(A.T) @ xt.T
  x_flat[n, t*64+h] = relu(xt[t,n] W1[h] + (0.9 u_fwd + 2.1 u_bwd)[t,n] W2[h])
  two MLP blocks (BN folded into weights on host) -> out (12,1536) per core.

Implementation notes:
  - KL adjacency compare computed SCALED by s[i] = Sum_t exp(x[t,i]) > 0:
      s*Ghat[i,j] = Sum_t ex[t,i]x[t,j] + s[i]*(-L[j]) + cmb[i]*1,
    L = ln(s), cmb = (0.5+L)*s - W, W = Sum_t ex*x.  K=34 operand stacks
    in f16 (f32r moving operands are SBUF-bandwidth-penalized on HW; f16
    runs 1 cyc/col; rows 12-31 zero for the 32-partition alignment rule):
    phat = [ex(12); 0; s@32; cmb@33], xs = [xt(12); 0; -L@32; 1@33].
    cmb keeps f32 intermediates (cancellation) and rounds only the result.
    No row duplication / explicit tile_position: one plain matmul per
    (i-block, chunk) per orientation.
  - A-orientation (ub side) compare on DVE is_gt -> exact 0/1 fp8e4 tiles.
    AT-orientation (uf side) on ScalarE Sign -> -1/0/1 fp8, with the affine
    correction folded into stage C:  yA = (yS + Sx)/2, rs = (rs' + N)/2
    -> uf = (yS + Sx)/(rs' + N)  (bias column applied on the psum copy).
  - products run as fp8 DoubleRow pair-matmuls (2 i-blocks = K=256 per
    call, 0.5 cyc/col): compare outputs land in [128,2,512] fp8 pair
    stacks; the stationary is a [128,2,48] fp8 transposed-x pack with a
    ones column at 32 (row/col sums land at psum partition 32).
  - reciprocals via reciprocal_approx_fast (18 bits, ~5x faster than
    reciprocal); K=1 matmuls broadcast 1/rs rows to 12 partitions.
  - x_flat is one K=76 matmul per (k,c) against the [W1;0;0.9W2;0;2.1W2]
    block-diagonal stack; MLP data path in f16 (tall moving operands at
    2B/row keep the PE off the SBUF-bandwidth wall).
  - the PE power governor (HAM) holds the PE at 1.2GHz whenever DVE/Act
    are concurrently busy, so the kernel is PHASE-SEPARATED: first all
    Ghat matmuls + compares for all chunks (PE capped at 1.2GHz by the
    compare load; A/S tiles stored in SBUF, 18KB/partition), then the
    DoubleRow product burst with DVE/Act quiet (PE ramps to 2.4GHz:
    216ns/512col measured), then the C chains (split across Act and
    DVE) overlapped with the x_flat/MLP rounds interleaved across the
    three 512-column chunks, which also run mostly at 2.4GHz.
    Stage-A transposes and sum matmuls cover the serial softmax-aug
    chain; H1 rounds for chunks 0-1 cover the last chunk's C drain.
  - all weights baked into the NEFF as inline consts (f16 blob, one DMA);
    per-call transfers are x in / out back only.  The jitted SPMD
    executable is cached across calls; weight changes detected by
    fingerprint.
"""

import numpy as np

import concourse.bass as bass
import concourse.bacc as bacc
import concourse.tile as tile
import concourse.mybir as mybir

F32 = mybir.dt.float32
F32R = mybir.dt.float32r
F16 = mybir.dt.float16
FP8 = mybir.dt.float8e4
AF = mybir.ActivationFunctionType
ALU = mybir.AluOpType
DR = mybir.MatmulPerfMode.DoubleRow

B, T, N, H, TH, HID2, TOUT = 8, 12, 1536, 64, 768, 128, 12
NT = N // 128          # 12 i-blocks
NP = NT // 2           # 6 DoubleRow pairs
NC = N // 512          # 3 column chunks

# ---- packed f16 weight blob column layout ----
O_ES = 0               # [76, 768] x_flat stack: W1/0.9W2/2.1W2 blockdiag @0/32/64
O_EW1 = 768            # 6 x 128 cols, rows 0-127
O_EPROJ = 1536         # 6 x 64 cols, rows 0-127
O_EW2 = 1920           # 128 cols
O_EW3 = 2048           # 64 cols
O_DW1 = 2112           # 128 cols, rows 0-63
O_DW2 = 2240           # 128 cols
O_DW3 = 2368           # 12 cols
O_DPROJ = 2380         # 12 cols, rows 0-63
O_EB1 = 2392           # f32 bias columns (pairs of f16 cols, bitcast)
O_EB2 = 2394
O_EBE = 2396
O_DB1 = 2398
O_DB2 = 2400
O_DBD = 2402
CW = 2404

_cache = {}


def _build_nc(wblob):
    nc = bacc.Bacc("TRN2", target_bir_lowering=False, debug=False)
    d = {}
    d["x"] = nc.declare_dram_parameter("x", [T, N], F32, isOutput=False)
    d["out"] = nc.declare_dram_parameter("out", [T, N], F32, isOutput=True)
    d["wb"] = nc.inline_tensor(wblob, name="wb")
    d["i12"] = nc.inline_tensor(np.eye(T, dtype=np.float16), name="i12")
    # zeros rows 0-20, ones row 21: one blob serves xs[12:34] (zeros + ones
    # row at 33) and phat[12:32] (zeros)
    zc = np.zeros((22, N), np.float16)
    zc[21, :] = 1.0
    d["zc"] = nc.inline_tensor(zc, name="zc")

    with tile.TileContext(nc) as tc:
        _kernel_body(tc, d)
    nc.compile()
    return nc


def _kernel_body(tc, d):
    nc = tc.nc
    CS = [slice(c * 512, (c + 1) * 512) for c in range(NC)]

    with tc.tile_pool(name="w", bufs=1) as w, tc.tile_pool(name="sb", bufs=1) as sb:

        def stile(name, shape, dt):
            return sb.tile(list(shape), dt, name=name, tag=name)

        # ---- per-call input + consts ----
        xin = stile("xin", (T, N), F32)
        nc.sync.dma_start(out=xin[:], in_=d["x"].ap())
        # xs/phat: Ghat j-/i-side stacks [xt(12); 0(20); -L@32; 1@33] /
        # [ex(12); 0(20); s@32; cmb@33], one f16 tile PER CHUNK so the
        # per-tile dependency on the aug DMA is per-chunk (B(0) starts on
        # chunk-0 aug instead of all three)
        xsC = [stile(f"xs{c}", (34, 512), F16) for c in range(NC)]
        phC = [stile(f"ph{c}", (34, 512), F16) for c in range(NC)]
        for c in range(NC):
            nc.gpsimd.dma_start(out=xsC[c][T:34, :], in_=d["zc"].ap()[:, CS[c]])
            nc.gpsimd.dma_start(out=phC[c][T:32, :], in_=d["zc"].ap()[0:20, CS[c]])
        wb = w.tile([128, CW], F16, name="wb", tag="wb")
        nc.scalar.dma_start(out=wb[:], in_=d["wb"].ap())
        i12 = w.tile([T, T], F16, name="i12", tag="i12")
        nc.gpsimd.dma_start(out=i12[:], in_=d["i12"].ap())

        ones12 = w.tile([T, 1], F16, name="ones12", tag="ones12")
        nc.vector.memset(ones12[:], 1.0)
        ones1 = w.tile([1, T], F32R, name="ones1", tag="ones1")
        nc.vector.memset(ones1[:].bitcast(F32), 1.0)
        # uf affine-correction bias: rows 0-11 = Sum_i xt[t,i], row 32 = N
        bSx = w.tile([33, 1], F32, name="bSx", tag="bSx")
        nc.vector.memset(bSx[32:33, :], float(N))
        # prewarm exp table under the input DMA
        warm = w.tile([1, 1], F32, name="warm", tag="warm")
        nc.vector.memset(warm[:], 1.0)
        nc.scalar.activation(warm[:], warm[:], AF.Exp)

        bias = {
            "eb1": wb[:, O_EB1:O_EB1 + 2].bitcast(F32),
            "eb2": wb[:, O_EB2:O_EB2 + 2].bitcast(F32),
            "ebe": wb[0:H, O_EBE:O_EBE + 2].bitcast(F32),
            "db1": wb[:, O_DB1:O_DB1 + 2].bitcast(F32),
            "db2": wb[:, O_DB2:O_DB2 + 2].bitcast(F32),
            "dbd": wb[0:TOUT, O_DBD:O_DBD + 2].bitcast(F32),
        }

        # fp8 transposed-x pair stack: [p, pair, member, col] col 32 = ones
        xtT = stile("xtT", (128, NP, 2, 48), FP8)
        nc.gpsimd.memset(xtT[:], 0.0)
        nc.vector.memset(xtT[:, :, :, 32:33], 1.0)
        # x_flat moving stack [xt(12); 0; uf@32; 0; ub@64] f16, one tile
        # per chunk so D(k,c) only depends on chunk c's uf/ub writes
        xdC = [stile(f"xd{c}", (76, 512), F16) for c in range(NC)]
        for c in range(NC):
            nc.gpsimd.memset(xdC[c][:], 0.0)
        for c in range(NC):
            nc.vector.tensor_copy(xsC[c][0:T, :], xin[:, CS[c]])

        # =========== Stage A ===========
        wx = stile("wx", (T, N), F16)
        cm32 = stile("cm32", (1, N), F32)
        L = stile("L", (1, N), F32)
        augP = stile("augP", (33, N), F16)
        with tc.tile_pool(name="pa", bufs=2, space="PSUM") as pa:
            for c in range(NC):
                nc.scalar.activation(phC[c][0:T, :], xin[:, CS[c]], AF.Exp)
            nc.scalar.activation(warm[:], warm[:], AF.Ln)   # hide Ln table load
            nc.vector.tensor_tensor(wx[:, CS[0]], phC[0][0:T, :],
                                    xsC[0][0:T, :], ALU.mult)

            # per-chunk [1,512] s/W psums (1 bank each, bufs=1) keep the
            # stage-A PSUM footprint at 3 banks so B(0)'s psG tiles get
            # untouched banks; the psT transposes cover chunk-0's serial
            # aug chain on Act/DVE
            psT = pa.tile([128, NT, T], F32, name="psT", tag="psT")
            psAs, psWs = [], []

            def emit_sums(c):
                psA = pa.tile([1, 512], F32, name="psA", tag="psA")
                psW = pa.tile([1, 512], F32, name="psW", tag="psW")
                nc.tensor.matmul(psA[:], ones12[:], phC[c][0:T, :],
                                 start=True, stop=True)
                nc.tensor.matmul(psW[:], ones12[:], wx[:, CS[c]],
                                 start=True, stop=True)
                psAs.append(psA); psWs.append(psW)

            def emit_chain(c):
                if c + 1 < NC:
                    nc.vector.tensor_tensor(wx[:, CS[c + 1]], phC[c + 1][0:T, :],
                                            xsC[c + 1][0:T, :], ALU.mult)
                psA, psW = psAs[c], psWs[c]
                nc.scalar.activation(L[:, CS[c]], psA[:], AF.Ln)
                nc.vector.tensor_scalar(xsC[c][32:33, :], L[:, CS[c]], -1.0,
                                        None, ALU.mult)
                nc.scalar.activation(augP[0:1, CS[c]], psA[:], AF.Identity)
                # cmb = (0.5 + L)*s - W, f32 intermediates, f16 final
                nc.vector.scalar_tensor_tensor(cm32[:, CS[c]], L[:, CS[c]], 0.5,
                                               psA[:], ALU.add, ALU.mult)
                nc.vector.tensor_tensor(augP[32:33, CS[c]], cm32[:, CS[c]],
                                        psW[:], ALU.subtract)
                eng = (nc.sync, nc.gpsimd, nc.scalar)[c]
                eng.dma_start(out=phC[c][32:34, :], in_=augP[0:33:32, CS[c]])

            emit_sums(0)
            emit_chain(0)
            emit_sums(1)
            emit_chain(1)
            emit_sums(2)
            emit_chain(2)
            for j in range(NT):
                nc.tensor.matmul(psT[:, j, :],
                                 xsC[j // 4][0:T, (j % 4) * 128:(j % 4 + 1) * 128],
                                 i12[:], start=True, stop=True)
            for j in range(NT):
                nc.vector.tensor_copy(xtT[:, j // 2, j % 2, 0:T], psT[:, j, :])
            for c in range(NC):
                nc.vector.tensor_copy(xdC[c][0:T, :], xin[:, CS[c]])
            nc.vector.tensor_reduce(bSx[0:T, :], xin[:],
                                    mybir.AxisListType.X, ALU.add)

        # =========== Stages B/C then MLP tail ===========
        vf = stile("vf", (T, N), F32)
        vb = stile("vb", (T, N), F32)
        rrA = stile("rrA", (1, N), F32R)
        rrB = stile("rrB", (1, N), F32R)
        zT = stile("zT", (128, 6, N), F16)
        h1 = stile("h1", (HID2, N), F16)
        h2 = stile("h2", (HID2, N), F16)
        xe = stile("xe", (H, N), F16)
        g1 = stile("g1", (HID2, N), F16)
        g2 = stile("g2", (HID2, N), F16)
        od = stile("od", (TOUT, N), F32)

        # full A / S fp8 pair-stacks, one tile per chunk (18KB/partition)
        Aall = [stile(f"A{c}", (128, NP, 2, 512), FP8) for c in range(NC)]
        Sall = [stile(f"S{c}", (128, NP, 2, 512), FP8) for c in range(NC)]

        # ---- Ghat + compares for ALL chunks (PE k4: the compare load
        # on DVE/Act holds the power governor down) ----
        with tc.tile_pool(name="pG", bufs=3, space="PSUM") as pG:
            if True:
                for c in range(NC):
                    for q in range(NP):
                        for m in range(2):
                            b = 2 * q + m
                            bc = b // 4
                            bs = slice((b % 4) * 128, (b % 4 + 1) * 128)
                            psG = pG.tile([128, 512], F32, name="psG", tag="g")
                            nc.tensor.matmul(psG[:], phC[bc][:, bs], xsC[c][:],
                                             start=True, stop=True)
                            nc.vector.tensor_scalar(Aall[c][:, q, m, :], psG[:],
                                                    0.0, None, ALU.is_gt)
                            psGT = pG.tile([128, 512], F32, name="psGT", tag="g")
                            nc.tensor.matmul(psGT[:], xsC[bc][:, bs], phC[c][:],
                                             start=True, stop=True)
                            nc.scalar.sign(Sall[c][:, q, m, :], psGT[:])

        with tc.tile_pool(name="pPA", bufs=2, space="PSUM") as pPA, \
             tc.tile_pool(name="pPB", bufs=2, space="PSUM") as pPB, \
             tc.tile_pool(name="pf", bufs=3, space="PSUM") as pf:

            c2q = []

            def emit_c2(c):
                # uf/ub broadcast matmuls + xd multiplies (PE + DVE);
                # the recip inputs are long done when these are emitted
                uB = pf.tile([T, 512], F32, name="uB", tag="ps")
                nc.tensor.matmul(uB[:], ones1[:], rrB[:, CS[c]],
                                 start=True, stop=True)
                nc.vector.tensor_tensor(xdC[c][32:44, :], vf[:, CS[c]],
                                        uB[:], ALU.mult)
                uA = pf.tile([T, 512], F32, name="uA", tag="ps")
                nc.tensor.matmul(uA[:], ones1[:], rrA[:, CS[c]],
                                 start=True, stop=True)
                nc.vector.tensor_tensor(xdC[c][64:76, :], vb[:, CS[c]],
                                        uA[:], ALU.mult)

            # ---- products burst (PE-only, DVE/Act quiet -> 2.4GHz) ----
            for c in range(NC):
                pA = pPA.tile([48, 512], F32, name="pA", tag="pA")
                pB = pPB.tile([48, 512], F32, name="pB", tag="pB")
                for q in range(NP):
                    nc.tensor.matmul(pA[:], xtT[:, q], Aall[c][:, q],
                                     start=(q == 0), stop=(q == NP - 1),
                                     perf_mode=DR)
                    nc.tensor.matmul(pB[:], xtT[:, q], Sall[c][:, q],
                                     start=(q == 0), stop=(q == NP - 1),
                                     perf_mode=DR)

                # C1(c): psum copies + reciprocals (no PE); vb/rrA on DVE
                # so the Act queue only carries the biased vf/rrB copies
                nc.vector.tensor_copy(vb[:, CS[c]], pA[0:T, :])
                nc.vector.tensor_copy(rrA[:, CS[c]], pA[32:33, :])
                nc.scalar.activation(vf[:, CS[c]], pB[0:T, :], AF.Identity,
                                     bias=bSx[0:T, :])
                nc.scalar.activation(rrB[:, CS[c]], pB[32:33, :],
                                     AF.Identity, bias=bSx[32:33, :])
                from concourse.dve_ops import (RECIP_APPROX_FAST_CONSTS as RC,
                                               RECIPROCAL_APPROX_FAST as RAF)
                nc.vector._custom_dve(RAF, out=rrA[:, CS[c]], in0=rrA[:, CS[c]],
                                      s0=RC["s0"], s1=RC["s1"], imm2=RC["imm2"])
                nc.vector._custom_dve(RAF, out=rrB[:, CS[c]], in0=rrB[:, CS[c]],
                                      s0=RC["s0"], s1=RC["s1"], imm2=RC["imm2"])
                if c2q:
                    emit_c2(c2q.pop(0))
                c2q.append(c)

            # ---- x_flat rounds for chunks 0..1, then H1(0)/H1(1) keep the
            # PE busy while C1(2)'s copies + reciprocals drain, then C2(2),
            # x_flat(2), H1(2) ----
            def emit_d(k, c):
                ps = pf.tile([128, 512], F32, name="psF", tag="ps")
                nc.tensor.matmul(ps[:], wb[0:76, O_ES + k * 128:O_ES + (k + 1) * 128],
                                 xdC[c][:], start=True, stop=True)
                if (k + c) % 2 == 0:
                    nc.scalar.activation(zT[:, k, CS[c]], ps[:], AF.Relu)
                else:
                    nc.vector.tensor_scalar(zT[:, k, CS[c]], ps[:], 0.0,
                                            None, ALU.max)

            def emit_h1(c):
                ps = pf.tile([HID2, 512], F32, name="psH1", tag="ps")
                for k in range(6):
                    nc.tensor.matmul(ps[:], wb[:, O_EW1 + k * 128:O_EW1 + (k + 1) * 128],
                                     zT[:, k, CS[c]], start=(k == 0), stop=(k == 5))
                if c % 2 == 0:
                    nc.scalar.activation(h1[:, CS[c]], ps[:], AF.Relu, bias=bias["eb1"])
                else:
                    nc.vector.tensor_scalar(h1[:, CS[c]], ps[:], bias["eb1"], 0.0,
                                            ALU.add, ALU.max)

            for k in range(6):
                for c in range(NC - 1):
                    emit_d(k, c)
            emit_h1(0)
            emit_h1(1)
            if c2q:
                emit_c2(c2q.pop(0))
            for k in range(6):
                emit_d(k, NC - 1)
            emit_h1(NC - 1)

            for c in range(NC):
                ps = pf.tile([HID2, 512], F32, name="psH2", tag="ps")
                nc.tensor.matmul(ps[:], wb[:, O_EW2:O_EW2 + HID2], h1[:, CS[c]],
                                 start=True, stop=True)
                if c % 2 == 1:
                    nc.scalar.activation(h2[:, CS[c]], ps[:], AF.Relu, bias=bias["eb2"])
                else:
                    nc.vector.tensor_scalar(h2[:, CS[c]], ps[:], bias["eb2"], 0.0,
                                            ALU.add, ALU.max)

            for c in range(NC):
                ps = pf.tile([H, 512], F32, name="psXe", tag="ps")
                nc.tensor.matmul(ps[:], wb[:, O_EW3:O_EW3 + H], h2[:, CS[c]],
                                 start=True, stop=False)
                for k in range(6):
                    nc.tensor.matmul(ps[:], wb[:, O_EPROJ + k * H:O_EPROJ + (k + 1) * H],
                                     zT[:, k, CS[c]], start=False, stop=(k == 5))
                if c % 2 == 0:
                    nc.scalar.activation(xe[:, CS[c]], ps[:], AF.Identity, bias=bias["ebe"])
                else:
                    nc.vector.tensor_scalar(xe[:, CS[c]], ps[:], bias["ebe"], None,
                                            ALU.add)

            for c in range(NC):
                ps = pf.tile([HID2, 512], F32, name="psG1", tag="ps")
                nc.tensor.matmul(ps[:], wb[0:H, O_DW1:O_DW1 + HID2], xe[:, CS[c]],
                                 start=True, stop=True)
                if c % 2 == 1:
                    nc.scalar.activation(g1[:, CS[c]], ps[:], AF.Relu, bias=bias["db1"])
                else:
                    nc.vector.tensor_scalar(g1[:, CS[c]], ps[:], bias["db1"], 0.0,
                                            ALU.add, ALU.max)

            for c in range(NC):
                ps = pf.tile([HID2, 512], F32, name="psG2", tag="ps")
                nc.tensor.matmul(ps[:], wb[:, O_DW2:O_DW2 + HID2], g1[:, CS[c]],
                                 start=True, stop=True)
                if c % 2 == 0:
                    nc.scalar.activation(g2[:, CS[c]], ps[:], AF.Relu, bias=bias["db2"])
                else:
                    nc.vector.tensor_scalar(g2[:, CS[c]], ps[:], bias["db2"], 0.0,
                                            ALU.add, ALU.max)

            for c in range(NC):
                ps = pf.tile([TOUT, 512], F32, name="psOd", tag="ps")
                nc.tensor.matmul(ps[:], wb[:, O_DW3:O_DW3 + TOUT], g2[:, CS[c]],
                                 start=True, stop=False)
                nc.tensor.matmul(ps[:], wb[0:H, O_DPROJ:O_DPROJ + TOUT], xe[:, CS[c]],
                                 start=False, stop=True)
                if c % 2 == 1:
                    nc.scalar.activation(od[:, CS[c]], ps[:], AF.Identity, bias=bias["dbd"])
                else:
                    nc.vector.tensor_scalar(od[:, CS[c]], ps[:], bias["dbd"], None,
                                            ALU.add)
                eng = (nc.gpsimd, nc.scalar, nc.sync)[c]
                eng.dma_start(out=d["out"].ap()[:, CS[c]], in_=od[:, CS[c]])


def _build_wblob(inputs):
    f32 = np.float32
    f16 = np.float16
    W1 = np.asarray(inputs["W1"], f32)[0]
    W2 = np.asarray(inputs["W2"], f32)[0]
    g = np.asarray(inputs["enc_bn_g"], f32); be = np.asarray(inputs["enc_bn_b"], f32)
    m = np.asarray(inputs["enc_bn_m"], f32); v = np.asarray(inputs["enc_bn_v"], f32)
    esc = g / np.sqrt(v + 1e-5)
    ew3 = np.asarray(inputs["enc_w3"], f32) * esc[None, :]
    eproj = np.asarray(inputs["enc_proj"], f32) * esc[None, :]
    ebe = np.asarray(inputs["enc_b3"], f32) * esc + (be - m * esc)
    g = np.asarray(inputs["dec_bn_g"], f32); bd = np.asarray(inputs["dec_bn_b"], f32)
    m = np.asarray(inputs["dec_bn_m"], f32); v = np.asarray(inputs["dec_bn_v"], f32)
    dsc = g / np.sqrt(v + 1e-5)
    dw3 = np.asarray(inputs["dec_w3"], f32) * dsc[None, :]
    dproj = np.asarray(inputs["dec_proj"], f32) * dsc[None, :]
    dbd = np.asarray(inputs["dec_b3"], f32) * dsc + (bd - m * dsc)

    wb = np.zeros((128, CW), f16)
    # x_flat stack: block-diagonal per t at rows 0/32/64
    for t in range(T):
        wb[t, O_ES + t * H:O_ES + (t + 1) * H] = W1.astype(f16)
        wb[32 + t, O_ES + t * H:O_ES + (t + 1) * H] = (0.9 * W2).astype(f16)
        wb[64 + t, O_ES + t * H:O_ES + (t + 1) * H] = (2.1 * W2).astype(f16)
    ew1 = np.asarray(inputs["enc_w1"], f32)
    for a in range(6):
        wb[:, O_EW1 + a * 128:O_EW1 + (a + 1) * 128] = ew1[a * 128:(a + 1) * 128, :].astype(f16)
        wb[:, O_EPROJ + a * H:O_EPROJ + (a + 1) * H] = eproj[a * 128:(a + 1) * 128, :].astype(f16)
    wb[:, O_EW2:O_EW2 + HID2] = np.asarray(inputs["enc_w2"], f32).astype(f16)
    wb[:, O_EW3:O_EW3 + H] = ew3.astype(f16)
    wb[0:H, O_DW1:O_DW1 + HID2] = np.asarray(inputs["dec_w1"], f32).astype(f16)
    wb[:, O_DW2:O_DW2 + HID2] = np.asarray(inputs["dec_w2"], f32).astype(f16)
    wb[:, O_DW3:O_DW3 + TOUT] = dw3.astype(f16)
    wb[0:H, O_DPROJ:O_DPROJ + TOUT] = dproj.astype(f16)

    def put_f32_col(off, rows, vals):
        col = np.zeros(128, f32)
        col[:rows] = vals
        wb[:, off:off + 2] = col.view(f16).reshape(128, 2)

    put_f32_col(O_EB1, HID2, np.asarray(inputs["enc_b1"], f32))
    put_f32_col(O_EB2, HID2, np.asarray(inputs["enc_b2"], f32))
    put_f32_col(O_EBE, H, ebe)
    put_f32_col(O_DB1, HID2, np.asarray(inputs["dec_b1"], f32))
    put_f32_col(O_DB2, HID2, np.asarray(inputs["dec_b2"], f32))
    put_f32_col(O_DBD, TOUT, dbd)
    return wb


def _weights_fp(inputs):
    """Content fingerprint of every non-x input (cheap; full-content hash)."""
    import hashlib
    h = hashlib.blake2b(digest_size=16)
    for k in sorted(inputs):
        if k == "x":
            continue
        a = np.ascontiguousarray(np.asarray(inputs[k]))
        h.update(k.encode())
        h.update(str(a.shape).encode())
        h.update(a.tobytes())
    return h.digest()


def _make_runner(nc):
    import jax
    from jax.sharding import Mesh, PartitionSpec, NamedSharding
    from jax.experimental.shard_map import shard_map
    from concourse.bass2jax import (_bass_exec_p, install_neuronx_cc_hook,
                                    partition_id_tensor)

    install_neuronx_cc_hook()
    partition_name = nc.partition_id_tensor.name if nc.partition_id_tensor else None

    in_names, out_names, out_avals, zero_shapes = [], [], [], []
    for alloc in nc.m.functions[0].allocations:
        if not isinstance(alloc, mybir.MemoryLocationSet):
            continue
        name = alloc.memorylocations[0].name
        if alloc.kind == "ExternalInput":
            if name != partition_name:
                in_names.append(name)
        elif alloc.kind == "ExternalOutput":
            out_names.append(name)
            shape = tuple(alloc.tensor_shape)
            dtype = mybir.dt.np(alloc.dtype)
            out_avals.append(jax.core.ShapedArray(shape, dtype))
            zero_shapes.append((shape, dtype))
    n_params = len(in_names)
    all_in_names = tuple(in_names + out_names + ([partition_name] if partition_name else []))

    def _body(*args):
        operands = list(args)
        if partition_name is not None:
            operands.append(partition_id_tensor())
        outs = _bass_exec_p.bind(
            *operands,
            out_avals=tuple(out_avals),
            in_names=all_in_names,
            out_names=tuple(out_names),
            lowering_input_output_aliases=(),
            sim_require_finite=True,
            sim_require_nnan=True,
            nc=nc,
        )
        return tuple(outs)

    devices = jax.devices()[:B]
    mesh = Mesh(np.asarray(devices), ("core",))
    nin = n_params + len(out_names)
    sharded = jax.jit(
        shard_map(_body, mesh=mesh, in_specs=(PartitionSpec("core"),) * nin,
                  out_specs=(PartitionSpec("core"),) * len(out_names), check_rep=False),
        keep_unused=True,
    )
    sh = NamedSharding(mesh, PartitionSpec("core"))
    zeros = [jax.device_put(np.zeros((B * s[0], *s[1:]), dt), sh)
             for (s, dt) in zero_shapes]
    return sharded, zeros


def _build_ctx(inputs):
    wb = _build_wblob(inputs)
    nc = _build_nc(wb)
    sharded, zeros = _make_runner(nc)
    return {"fp": _weights_fp(inputs), "nc": nc, "sharded": sharded, "zeros": zeros,
            "ids": tuple(id(inputs[k]) for k in sorted(inputs) if k != "x")}


def kernel(**inputs) -> np.ndarray:
    ctx = _cache.get("ctx")
    if ctx is not None:
        ids = tuple(id(inputs[k]) for k in sorted(inputs) if k != "x")
        if ids != ctx["ids"]:
            if _weights_fp(inputs) == ctx["fp"]:
                ctx["ids"] = ids
            else:
                ctx = None
    if ctx is None:
        ctx = _build_ctx(inputs)
        _cache["ctx"] = ctx

    x = np.asarray(inputs["x"], np.float32).reshape(B * T, N)
    out = ctx["sharded"](x, *ctx["zeros"])[0]
    return np.asarray(out).reshape(B, TOUT, N, 1).astype(np.float32, copy=False)


# revision 42
# speedup vs baseline: 1.0099x; 1.0099x over previous
"""Trainium2 Bass kernel for AdaBiDiff GNN message passing.

Data parallel over batch B=8, one batch element per core.  Per core:
  xt (12,1536) -> softmax over t -> p, logp (t-major)
  kl[i,j] = rowterm[i] - sum_t p[i,t] logp[j,t];  A = (kl < 0.5)
  u_fwd = rownorm(A) @ xt.T;  u_bwd = rownorm(A.T) @ xt.T
  x_flat[n, t*64+h] = relu(xt[t,n] W1[h] + (0.9 u_fwd + 2.1 u_bwd)[t,n] W2[h])
  two MLP blocks (BN folded into weights on host) -> out (12,1536) per core.

Implementation notes:
  - KL adjacency compare computed SCALED by s[i] = Sum_t exp(x[t,i]) > 0:
      s*Ghat[i,j] = Sum_t ex[t,i]x[t,j] + s[i]*(-L[j]) + cmb[i]*1,
    L = ln(s), cmb = (0.5+L)*s - W, W = Sum_t ex*x.  K=34 operand stacks
    in f16 (f32r moving operands are SBUF-bandwidth-penalized on HW; f16
    runs 1 cyc/col; rows 12-31 zero for the 32-partition alignment rule):
    phat = [ex(12); 0; s@32; cmb@33], xs = [xt(12); 0; -L@32; 1@33].
    cmb keeps f32 intermediates (cancellation) and rounds only the result.
    No row duplication / explicit tile_position: one plain matmul per
    (i-block, chunk) per orientation.
  - A-orientation (ub side) compare on DVE is_gt -> exact 0/1 fp8e4 tiles.
    AT-orientation (uf side) on ScalarE Sign -> -1/0/1 fp8, with the affine
    correction folded into stage C:  yA = (yS + Sx)/2, rs = (rs' + N)/2
    -> uf = (yS + Sx)/(rs' + N)  (bias column applied on the psum copy).
  - products run as fp8 DoubleRow pair-matmuls (2 i-blocks = K=256 per
    call, 0.5 cyc/col): compare outputs land in [128,2,512] fp8 pair
    stacks; the stationary is a [128,2,48] fp8 transposed-x pack with a
    ones column at 32 (row/col sums land at psum partition 32).
  - reciprocals via reciprocal_approx_fast (18 bits, ~5x faster than
    reciprocal); K=1 matmuls broadcast 1/rs rows to 12 partitions.
  - x_flat is one K=76 matmul per (k,c) against the [W1;0;0.9W2;0;2.1W2]
    block-diagonal stack; MLP data path in f16 (tall moving operands at
    2B/row keep the PE off the SBUF-bandwidth wall).
  - the PE power governor (HAM) holds the PE at 1.2GHz whenever DVE/Act
    are concurrently busy, so the kernel is PHASE-SEPARATED: first all
    Ghat matmuls + compares for all chunks (PE capped at 1.2GHz by the
    compare load; A/S tiles stored in SBUF, 18KB/partition), then the
    DoubleRow product burst with DVE/Act quiet (PE ramps to 2.4GHz:
    216ns/512col measured), then the C chains (split across Act and
    DVE) overlapped with the x_flat/MLP rounds interleaved across the
    three 512-column chunks, which also run mostly at 2.4GHz.
    Stage-A transposes and sum matmuls cover the serial softmax-aug
    chain; H1 rounds for chunks 0-1 cover the last chunk's C drain.
  - all weights baked into the NEFF as inline consts (f16 blob, one DMA);
    per-call transfers are x in / out back only.  The jitted SPMD
    executable is cached across calls; weight changes detected by
    fingerprint.
"""

import numpy as np

import concourse.bass as bass
import concourse.bacc as bacc
import concourse.tile as tile
import concourse.mybir as mybir

F32 = mybir.dt.float32
F32R = mybir.dt.float32r
F16 = mybir.dt.float16
FP8 = mybir.dt.float8e4
AF = mybir.ActivationFunctionType
ALU = mybir.AluOpType
DR = mybir.MatmulPerfMode.DoubleRow

B, T, N, H, TH, HID2, TOUT = 8, 12, 1536, 64, 768, 128, 12
NT = N // 128          # 12 i-blocks
NP = NT // 2           # 6 DoubleRow pairs
NC = N // 512          # 3 column chunks

# ---- packed f16 weight blob column layout ----
O_ES = 0               # [76, 768] x_flat stack: W1/0.9W2/2.1W2 blockdiag @0/32/64
O_EW1 = 768            # 6 x 128 cols, rows 0-127
O_EPROJ = 1536         # 6 x 64 cols, rows 0-127
O_EW2 = 1920           # 128 cols
O_EW3 = 2048           # 64 cols
O_DW1 = 2112           # 128 cols, rows 0-63
O_DW2 = 2240           # 128 cols
O_DW3 = 2368           # 12 cols
O_DPROJ = 2380         # 12 cols, rows 0-63
O_EB1 = 2392           # f32 bias columns (pairs of f16 cols, bitcast)
O_EB2 = 2394
O_EBE = 2396
O_DB1 = 2398
O_DB2 = 2400
O_DBD = 2402
CW = 2404

_cache = {}


def _build_nc(wblob):
    nc = bacc.Bacc("TRN2", target_bir_lowering=False, debug=False)
    d = {}
    d["x"] = nc.declare_dram_parameter("x", [T, N], F32, isOutput=False)
    d["out"] = nc.declare_dram_parameter("out", [T, N], F32, isOutput=True)
    d["wb"] = nc.inline_tensor(wblob, name="wb")
    d["i12"] = nc.inline_tensor(np.eye(T, dtype=np.float16), name="i12")
    # zeros rows 0-20, ones row 21: one blob serves xs[12:34] (zeros + ones
    # row at 33) and phat[12:32] (zeros)
    zc = np.zeros((22, N), np.float16)
    zc[21, :] = 1.0
    d["zc"] = nc.inline_tensor(zc, name="zc")

    with tile.TileContext(nc) as tc:
        _kernel_body(tc, d)
    nc.compile()
    return nc


def _kernel_body(tc, d):
    nc = tc.nc
    CS = [slice(c * 512, (c + 1) * 512) for c in range(NC)]

    with tc.tile_pool(name="w", bufs=1) as w, tc.tile_pool(name="sb", bufs=1) as sb:

        def stile(name, shape, dt):
            return sb.tile(list(shape), dt, name=name, tag=name)

        # ---- per-call input + consts ----
        xin = stile("xin", (T, N), F32)
        nc.sync.dma_start(out=xin[:], in_=d["x"].ap())
        # xs/phat: Ghat j-/i-side stacks [xt(12); 0(20); -L@32; 1@33] /
        # [ex(12); 0(20); s@32; cmb@33], one f16 tile PER CHUNK so the
        # per-tile dependency on the aug DMA is per-chunk (B(0) starts on
        # chunk-0 aug instead of all three)
        xsC = [stile(f"xs{c}", (34, 512), F16) for c in range(NC)]
        phC = [stile(f"ph{c}", (34, 512), F16) for c in range(NC)]
        for c in range(NC):
            nc.gpsimd.dma_start(out=xsC[c][T:34, :], in_=d["zc"].ap()[:, CS[c]])
            nc.gpsimd.dma_start(out=phC[c][T:32, :], in_=d["zc"].ap()[0:20, CS[c]])
        wb = w.tile([128, CW], F16, name="wb", tag="wb")
        nc.scalar.dma_start(out=wb[:], in_=d["wb"].ap())
        i12 = w.tile([T, T], F16, name="i12", tag="i12")
        nc.gpsimd.dma_start(out=i12[:], in_=d["i12"].ap())

        ones12 = w.tile([T, 1], F16, name="ones12", tag="ones12")
        nc.vector.memset(ones12[:], 1.0)
        ones1 = w.tile([1, T], F32R, name="ones1", tag="ones1")
        nc.vector.memset(ones1[:].bitcast(F32), 1.0)
        # uf affine-correction bias: rows 0-11 = Sum_i xt[t,i], row 32 = N
        bSx = w.tile([33, 1], F32, name="bSx", tag="bSx")
        nc.vector.memset(bSx[32:33, :], float(N))
        # prewarm exp table under the input DMA
        warm = w.tile([1, 1], F32, name="warm", tag="warm")
        nc.vector.memset(warm[:], 1.0)
        nc.scalar.activation(warm[:], warm[:], AF.Exp)

        bias = {
            "eb1": wb[:, O_EB1:O_EB1 + 2].bitcast(F32),
            "eb2": wb[:, O_EB2:O_EB2 + 2].bitcast(F32),
            "ebe": wb[0:H, O_EBE:O_EBE + 2].bitcast(F32),
            "db1": wb[:, O_DB1:O_DB1 + 2].bitcast(F32),
            "db2": wb[:, O_DB2:O_DB2 + 2].bitcast(F32),
            "dbd": wb[0:TOUT, O_DBD:O_DBD + 2].bitcast(F32),
        }

        # fp8 transposed-x pair stack: [p, pair, member, col] col 32 = ones
        xtT = stile("xtT", (128, NP, 2, 48), FP8)
        nc.gpsimd.memset(xtT[:], 0.0)
        nc.vector.memset(xtT[:, :, :, 32:33], 1.0)
        # x_flat moving stack [xt(12); 0; uf@32; 0; ub@64] f16, one tile
        # per chunk so D(k,c) only depends on chunk c's uf/ub writes
        xdC = [stile(f"xd{c}", (76, 512), F16) for c in range(NC)]
        for c in range(NC):
            nc.gpsimd.memset(xdC[c][:], 0.0)
        for c in range(NC):
            nc.vector.tensor_copy(xsC[c][0:T, :], xin[:, CS[c]])

        # =========== Stage A ===========
        wx = stile("wx", (T, N), F16)
        cm32 = stile("cm32", (1, N), F32)
        L = stile("L", (1, N), F32)
        augP = stile("augP", (33, N), F16)
        with tc.tile_pool(name="pa", bufs=2, space="PSUM") as pa, \
             tc.tile_pool(name="pt", bufs=1, space="PSUM") as pt:
            for c in range(NC):
                nc.scalar.activation(phC[c][0:T, :], xin[:, CS[c]], AF.Exp)
            nc.scalar.activation(warm[:], warm[:], AF.Ln)   # hide Ln table load
            nc.vector.tensor_tensor(wx[:, CS[0]], phC[0][0:T, :],
                                    xsC[0][0:T, :], ALU.mult)

            # per-chunk [1,512] s/W psums (1 bank each, bufs=1) keep the
            # stage-A PSUM footprint at 3 banks so B(0)'s psG tiles get
            # untouched banks; the psT transposes cover chunk-0's serial
            # aug chain on Act/DVE
            psT = pt.tile([128, NT, T], F32, name="psT", tag="psT")
            psAs, psWs = [], []

            def emit_sums(c):
                psA = pa.tile([1, 512], F32, name="psA", tag="psA")
                psW = pa.tile([1, 512], F32, name="psW", tag="psW")
                nc.tensor.matmul(psA[:], ones12[:], phC[c][0:T, :],
                                 start=True, stop=True)
                nc.tensor.matmul(psW[:], ones12[:], wx[:, CS[c]],
                                 start=True, stop=True)
                psAs.append(psA); psWs.append(psW)

            def emit_chain(c):
                if c + 1 < NC:
                    nc.vector.tensor_tensor(wx[:, CS[c + 1]], phC[c + 1][0:T, :],
                                            xsC[c + 1][0:T, :], ALU.mult)
                psA, psW = psAs[c], psWs[c]
                nc.scalar.activation(L[:, CS[c]], psA[:], AF.Ln)
                nc.vector.tensor_scalar(xsC[c][32:33, :], L[:, CS[c]], -1.0,
                                        None, ALU.mult)
                nc.scalar.activation(augP[0:1, CS[c]], psA[:], AF.Identity)
                # cmb = (0.5 + L)*s - W, f32 intermediates, f16 final
                nc.vector.scalar_tensor_tensor(cm32[:, CS[c]], L[:, CS[c]], 0.5,
                                               psA[:], ALU.add, ALU.mult)
                nc.vector.tensor_tensor(augP[32:33, CS[c]], cm32[:, CS[c]],
                                        psW[:], ALU.subtract)
                eng = (nc.sync, nc.gpsimd, nc.scalar)[c]
                eng.dma_start(out=phC[c][32:34, :], in_=augP[0:33:32, CS[c]])

            emit_sums(0)
            emit_chain(0)
            emit_sums(1)
            emit_chain(1)
            emit_sums(2)
            emit_chain(2)
            for j in range(NT):
                nc.tensor.matmul(psT[:, j, :],
                                 xsC[j // 4][0:T, (j % 4) * 128:(j % 4 + 1) * 128],
                                 i12[:], start=True, stop=True)
            for j in range(NT):
                nc.vector.tensor_copy(xtT[:, j // 2, j % 2, 0:T], psT[:, j, :])
            for c in range(NC):
                nc.vector.tensor_copy(xdC[c][0:T, :], xin[:, CS[c]])
            nc.vector.tensor_reduce(bSx[0:T, :], xin[:],
                                    mybir.AxisListType.X, ALU.add)

        # =========== Stages B/C then MLP tail ===========
        vf = stile("vf", (T, N), F32)
        vb = stile("vb", (T, N), F32)
        rrA = stile("rrA", (1, N), F32R)
        rrB = stile("rrB", (1, N), F32R)
        zT = stile("zT", (128, 6, N), F16)
        h1 = stile("h1", (HID2, N), F16)
        h2 = stile("h2", (HID2, N), F16)
        xe = stile("xe", (H, N), F16)
        g1 = stile("g1", (HID2, N), F16)
        g2 = stile("g2", (HID2, N), F16)
        od = stile("od", (TOUT, N), F32)

        # full A / S fp8 pair-stacks, one tile per chunk (18KB/partition)
        Aall = [stile(f"A{c}", (128, NP, 2, 512), FP8) for c in range(NC)]
        Sall = [stile(f"S{c}", (128, NP, 2, 512), FP8) for c in range(NC)]

        # ---- Ghat + compares for ALL chunks (PE k4: the compare load
        # on DVE/Act holds the power governor down) ----
        with tc.tile_pool(name="pG", bufs=3, space="PSUM") as pG:
            if True:
                for c in range(NC):
                    for q in range(NP):
                        for m in range(2):
                            b = 2 * q + m
                            bc = b // 4
                            bs = slice((b % 4) * 128, (b % 4 + 1) * 128)
                            psG = pG.tile([128, 512], F32, name="psG", tag="g")
                            nc.tensor.matmul(psG[:], phC[bc][:, bs], xsC[c][:],
                                             start=True, stop=True)
                            nc.vector.tensor_scalar(Aall[c][:, q, m, :], psG[:],
                                                    0.0, None, ALU.is_gt)
                            psGT = pG.tile([128, 512], F32, name="psGT", tag="g")
                            nc.tensor.matmul(psGT[:], xsC[bc][:, bs], phC[c][:],
                                             start=True, stop=True)
                            nc.scalar.sign(Sall[c][:, q, m, :], psGT[:])

        with tc.tile_pool(name="pPA", bufs=2, space="PSUM") as pPA, \
             tc.tile_pool(name="pPB", bufs=2, space="PSUM") as pPB, \
             tc.tile_pool(name="pf", bufs=3, space="PSUM") as pf:

            c2q = []

            def emit_c2(c):
                # uf/ub broadcast matmuls + xd multiplies (PE + DVE);
                # the recip inputs are long done when these are emitted
                uB = pf.tile([T, 512], F32, name="uB", tag="ps")
                nc.tensor.matmul(uB[:], ones1[:], rrB[:, CS[c]],
                                 start=True, stop=True)
                nc.vector.tensor_tensor(xdC[c][32:44, :], vf[:, CS[c]],
                                        uB[:], ALU.mult)
                uA = pf.tile([T, 512], F32, name="uA", tag="ps")
                nc.tensor.matmul(uA[:], ones1[:], rrA[:, CS[c]],
                                 start=True, stop=True)
                nc.vector.tensor_tensor(xdC[c][64:76, :], vb[:, CS[c]],
                                        uA[:], ALU.mult)

            # ---- products burst (PE-only, DVE/Act quiet -> 2.4GHz) ----
            for c in range(NC):
                pA = pPA.tile([48, 512], F32, name="pA", tag="pA")
                pB = pPB.tile([48, 512], F32, name="pB", tag="pB")
                for q in range(NP):
                    nc.tensor.matmul(pA[:], xtT[:, q], Aall[c][:, q],
                                     start=(q == 0), stop=(q == NP - 1),
                                     perf_mode=DR)
                    nc.tensor.matmul(pB[:], xtT[:, q], Sall[c][:, q],
                                     start=(q == 0), stop=(q == NP - 1),
                                     perf_mode=DR)

                # C1(c): psum copies + reciprocals (no PE); vb/rrA on DVE
                # so the Act queue only carries the biased vf/rrB copies
                nc.vector.tensor_copy(vb[:, CS[c]], pA[0:T, :])
                nc.vector.tensor_copy(rrA[:, CS[c]], pA[32:33, :])
                nc.scalar.activation(vf[:, CS[c]], pB[0:T, :], AF.Identity,
                                     bias=bSx[0:T, :])
                nc.scalar.activation(rrB[:, CS[c]], pB[32:33, :],
                                     AF.Identity, bias=bSx[32:33, :])
                from concourse.dve_ops import (RECIP_APPROX_FAST_CONSTS as RC,
                                               RECIPROCAL_APPROX_FAST as RAF)
                nc.vector._custom_dve(RAF, out=rrA[:, CS[c]], in0=rrA[:, CS[c]],
                                      s0=RC["s0"], s1=RC["s1"], imm2=RC["imm2"])
                nc.vector._custom_dve(RAF, out=rrB[:, CS[c]], in0=rrB[:, CS[c]],
                                      s0=RC["s0"], s1=RC["s1"], imm2=RC["imm2"])
                if c2q:
                    emit_c2(c2q.pop(0))
                c2q.append(c)

            # ---- x_flat rounds for chunks 0..1, then H1(0)/H1(1) keep the
            # PE busy while C1(2)'s copies + reciprocals drain, then C2(2),
            # x_flat(2), H1(2) ----
            def emit_d(k, c):
                ps = pf.tile([128, 512], F32, name="psF", tag="ps")
                nc.tensor.matmul(ps[:], wb[0:76, O_ES + k * 128:O_ES + (k + 1) * 128],
                                 xdC[c][:], start=True, stop=True)
                if (k + c) % 2 == 0:
                    nc.scalar.activation(zT[:, k, CS[c]], ps[:], AF.Relu)
                else:
                    nc.vector.tensor_scalar(zT[:, k, CS[c]], ps[:], 0.0,
                                            None, ALU.max)

            def emit_h1(c):
                ps = pf.tile([HID2, 512], F32, name="psH1", tag="ps")
                for k in range(6):
                    nc.tensor.matmul(ps[:], wb[:, O_EW1 + k * 128:O_EW1 + (k + 1) * 128],
                                     zT[:, k, CS[c]], start=(k == 0), stop=(k == 5))
                if c % 2 == 0:
                    nc.scalar.activation(h1[:, CS[c]], ps[:], AF.Relu, bias=bias["eb1"])
                else:
                    nc.vector.tensor_scalar(h1[:, CS[c]], ps[:], bias["eb1"], 0.0,
                                            ALU.add, ALU.max)

            for k in range(6):
                for c in range(NC - 1):
                    emit_d(k, c)
            emit_h1(0)
            emit_h1(1)
            if c2q:
                emit_c2(c2q.pop(0))
            for k in range(6):
                emit_d(k, NC - 1)
            emit_h1(NC - 1)

            for c in range(NC):
                ps = pf.tile([HID2, 512], F32, name="psH2", tag="ps")
                nc.tensor.matmul(ps[:], wb[:, O_EW2:O_EW2 + HID2], h1[:, CS[c]],
                                 start=True, stop=True)
                if c % 2 == 1:
                    nc.scalar.activation(h2[:, CS[c]], ps[:], AF.Relu, bias=bias["eb2"])
                else:
                    nc.vector.tensor_scalar(h2[:, CS[c]], ps[:], bias["eb2"], 0.0,
                                            ALU.add, ALU.max)

            for c in range(NC):
                ps = pf.tile([H, 512], F32, name="psXe", tag="ps")
                nc.tensor.matmul(ps[:], wb[:, O_EW3:O_EW3 + H], h2[:, CS[c]],
                                 start=True, stop=False)
                for k in range(6):
                    nc.tensor.matmul(ps[:], wb[:, O_EPROJ + k * H:O_EPROJ + (k + 1) * H],
                                     zT[:, k, CS[c]], start=False, stop=(k == 5))
                if c % 2 == 0:
                    nc.scalar.activation(xe[:, CS[c]], ps[:], AF.Identity, bias=bias["ebe"])
                else:
                    nc.vector.tensor_scalar(xe[:, CS[c]], ps[:], bias["ebe"], None,
                                            ALU.add)

            for c in range(NC):
                ps = pf.tile([HID2, 512], F32, name="psG1", tag="ps")
                nc.tensor.matmul(ps[:], wb[0:H, O_DW1:O_DW1 + HID2], xe[:, CS[c]],
                                 start=True, stop=True)
                if c % 2 == 1:
                    nc.scalar.activation(g1[:, CS[c]], ps[:], AF.Relu, bias=bias["db1"])
                else:
                    nc.vector.tensor_scalar(g1[:, CS[c]], ps[:], bias["db1"], 0.0,
                                            ALU.add, ALU.max)

            for c in range(NC):
                ps = pf.tile([HID2, 512], F32, name="psG2", tag="ps")
                nc.tensor.matmul(ps[:], wb[:, O_DW2:O_DW2 + HID2], g1[:, CS[c]],
                                 start=True, stop=True)
                if c % 2 == 0:
                    nc.scalar.activation(g2[:, CS[c]], ps[:], AF.Relu, bias=bias["db2"])
                else:
                    nc.vector.tensor_scalar(g2[:, CS[c]], ps[:], bias["db2"], 0.0,
                                            ALU.add, ALU.max)

            for c in range(NC):
                ps = pf.tile([TOUT, 512], F32, name="psOd", tag="ps")
                nc.tensor.matmul(ps[:], wb[:, O_DW3:O_DW3 + TOUT], g2[:, CS[c]],
                                 start=True, stop=False)
                nc.tensor.matmul(ps[:], wb[0:H, O_DPROJ:O_DPROJ + TOUT], xe[:, CS[c]],
                                 start=False, stop=True)
                if c % 2 == 1:
                    nc.scalar.activation(od[:, CS[c]], ps[:], AF.Identity, bias=bias["dbd"])
                else:
                    nc.vector.tensor_scalar(od[:, CS[c]], ps[:], bias["dbd"], None,
                                            ALU.add)
                eng = (nc.gpsimd, nc.scalar, nc.sync)[c]
                eng.dma_start(out=d["out"].ap()[:, CS[c]], in_=od[:, CS[c]])


def _build_wblob(inputs):
    f32 = np.float32
    f16 = np.float16
    W1 = np.asarray(inputs["W1"], f32)[0]
    W2 = np.asarray(inputs["W2"], f32)[0]
    g = np.asarray(inputs["enc_bn_g"], f32); be = np.asarray(inputs["enc_bn_b"], f32)
    m = np.asarray(inputs["enc_bn_m"], f32); v = np.asarray(inputs["enc_bn_v"], f32)
    esc = g / np.sqrt(v + 1e-5)
    ew3 = np.asarray(inputs["enc_w3"], f32) * esc[None, :]
    eproj = np.asarray(inputs["enc_proj"], f32) * esc[None, :]
    ebe = np.asarray(inputs["enc_b3"], f32) * esc + (be - m * esc)
    g = np.asarray(inputs["dec_bn_g"], f32); bd = np.asarray(inputs["dec_bn_b"], f32)
    m = np.asarray(inputs["dec_bn_m"], f32); v = np.asarray(inputs["dec_bn_v"], f32)
    dsc = g / np.sqrt(v + 1e-5)
    dw3 = np.asarray(inputs["dec_w3"], f32) * dsc[None, :]
    dproj = np.asarray(inputs["dec_proj"], f32) * dsc[None, :]
    dbd = np.asarray(inputs["dec_b3"], f32) * dsc + (bd - m * dsc)

    wb = np.zeros((128, CW), f16)
    # x_flat stack: block-diagonal per t at rows 0/32/64
    for t in range(T):
        wb[t, O_ES + t * H:O_ES + (t + 1) * H] = W1.astype(f16)
        wb[32 + t, O_ES + t * H:O_ES + (t + 1) * H] = (0.9 * W2).astype(f16)
        wb[64 + t, O_ES + t * H:O_ES + (t + 1) * H] = (2.1 * W2).astype(f16)
    ew1 = np.asarray(inputs["enc_w1"], f32)
    for a in range(6):
        wb[:, O_EW1 + a * 128:O_EW1 + (a + 1) * 128] = ew1[a * 128:(a + 1) * 128, :].astype(f16)
        wb[:, O_EPROJ + a * H:O_EPROJ + (a + 1) * H] = eproj[a * 128:(a + 1) * 128, :].astype(f16)
    wb[:, O_EW2:O_EW2 + HID2] = np.asarray(inputs["enc_w2"], f32).astype(f16)
    wb[:, O_EW3:O_EW3 + H] = ew3.astype(f16)
    wb[0:H, O_DW1:O_DW1 + HID2] = np.asarray(inputs["dec_w1"], f32).astype(f16)
    wb[:, O_DW2:O_DW2 + HID2] = np.asarray(inputs["dec_w2"], f32).astype(f16)
    wb[:, O_DW3:O_DW3 + TOUT] = dw3.astype(f16)
    wb[0:H, O_DPROJ:O_DPROJ + TOUT] = dproj.astype(f16)

    def put_f32_col(off, rows, vals):
        col = np.zeros(128, f32)
        col[:rows] = vals
        wb[:, off:off + 2] = col.view(f16).reshape(128, 2)

    put_f32_col(O_EB1, HID2, np.asarray(inputs["enc_b1"], f32))
    put_f32_col(O_EB2, HID2, np.asarray(inputs["enc_b2"], f32))
    put_f32_col(O_EBE, H, ebe)
    put_f32_col(O_DB1, HID2, np.asarray(inputs["dec_b1"], f32))
    put_f32_col(O_DB2, HID2, np.asarray(inputs["dec_b2"], f32))
    put_f32_col(O_DBD, TOUT, dbd)
    return wb


def _weights_fp(inputs):
    """Content fingerprint of every non-x input (cheap; full-content hash)."""
    import hashlib
    h = hashlib.blake2b(digest_size=16)
    for k in sorted(inputs):
        if k == "x":
            continue
        a = np.ascontiguousarray(np.asarray(inputs[k]))
        h.update(k.encode())
        h.update(str(a.shape).encode())
        h.update(a.tobytes())
    return h.digest()


def _make_runner(nc):
    import jax
    from jax.sharding import Mesh, PartitionSpec, NamedSharding
    from jax.experimental.shard_map import shard_map
    from concourse.bass2jax import (_bass_exec_p, install_neuronx_cc_hook,
                                    partition_id_tensor)

    install_neuronx_cc_hook()
    partition_name = nc.partition_id_tensor.name if nc.partition_id_tensor else None

    in_names, out_names, out_avals, zero_shapes = [], [], [], []
    for alloc in nc.m.functions[0].allocations:
        if not isinstance(alloc, mybir.MemoryLocationSet):
            continue
        name = alloc.memorylocations[0].name
        if alloc.kind == "ExternalInput":
            if name != partition_name:
                in_names.append(name)
        elif alloc.kind == "ExternalOutput":
            out_names.append(name)
            shape = tuple(alloc.tensor_shape)
            dtype = mybir.dt.np(alloc.dtype)
            out_avals.append(jax.core.ShapedArray(shape, dtype))
            zero_shapes.append((shape, dtype))
    n_params = len(in_names)
    all_in_names = tuple(in_names + out_names + ([partition_name] if partition_name else []))

    def _body(*args):
        operands = list(args)
        if partition_name is not None:
            operands.append(partition_id_tensor())
        outs = _bass_exec_p.bind(
            *operands,
            out_avals=tuple(out_avals),
            in_names=all_in_names,
            out_names=tuple(out_names),
            lowering_input_output_aliases=(),
            sim_require_finite=True,
            sim_require_nnan=True,
            nc=nc,
        )
        return tuple(outs)

    devices = jax.devices()[:B]
    mesh = Mesh(np.asarray(devices), ("core",))
    nin = n_params + len(out_names)
    sharded = jax.jit(
        shard_map(_body, mesh=mesh, in_specs=(PartitionSpec("core"),) * nin,
                  out_specs=(PartitionSpec("core"),) * len(out_names), check_rep=False),
        keep_unused=True,
    )
    sh = NamedSharding(mesh, PartitionSpec("core"))
    zeros = [jax.device_put(np.zeros((B * s[0], *s[1:]), dt), sh)
             for (s, dt) in zero_shapes]
    return sharded, zeros


def _build_ctx(inputs):
    wb = _build_wblob(inputs)
    nc = _build_nc(wb)
    sharded, zeros = _make_runner(nc)
    return {"fp": _weights_fp(inputs), "nc": nc, "sharded": sharded, "zeros": zeros,
            "ids": tuple(id(inputs[k]) for k in sorted(inputs) if k != "x")}


def kernel(**inputs) -> np.ndarray:
    ctx = _cache.get("ctx")
    if ctx is not None:
        ids = tuple(id(inputs[k]) for k in sorted(inputs) if k != "x")
        if ids != ctx["ids"]:
            if _weights_fp(inputs) == ctx["fp"]:
                ctx["ids"] = ids
            else:
                ctx = None
    if ctx is None:
        ctx = _build_ctx(inputs)
        _cache["ctx"] = ctx

    x = np.asarray(inputs["x"], np.float32).reshape(B * T, N)
    out = ctx["sharded"](x, *ctx["zeros"])[0]
    return np.asarray(out).reshape(B, TOUT, N, 1).astype(np.float32, copy=False)
